# revision 48
# baseline (speedup 1.0000x reference)
"""Multi-head causal attention (B=4, S=2048, H=1024, NH=16) on 8 trn2 cores.

Head-sharded tensor parallelism: core i computes heads {2i, 2i+1}.  Each core
runs projections for its 2 heads (fp32r matmuls), causal flash-style attention
in a transposed orientation (scores S^T[k,q] so the P@V contraction needs no
transpose of P), and a partial output projection over its 128 channels.  The
8 partial outputs are summed on the host (the tensor-parallel all-reduce),
plus the output bias.
"""
import numpy as np

import concourse.bacc as bacc
import concourse.tile as tile
from concourse import mybir
from concourse.bass_utils import run_bass_kernel_spmd

F32 = mybir.dt.float32
F32R = mybir.dt.float32r
AF = mybir.ActivationFunctionType

B, S, H, NH = 4, 2048, 1024, 16
HD = H // NH            # 64
NCORES = 8
HPC = NH // NCORES      # 2 heads per core
C = HPC * HD            # 128 channels per core
SCALE = 1.0 / np.sqrt(HD)

QT_W = 256              # q-tile width (columns of S^T tiles)
KC = 128                # k-chunk (contraction tile for P@V)
N_QT = S // QT_W        # 8
N_KC = S // KC          # 16
N_HC = H // 128         # 8 contraction chunks for projections
N_ST = 4                # s-tiles of 512 for projections

_CACHE = {}
PHASES = ("proj", "vtrans", "attn", "oproj")
PROJ_PRIO = 0


def _build_nc():
    nc = bacc.Bacc(name="mha_tp")
    xt_d = nc.dram_tensor("xt", [B, H, S], F32R, kind="ExternalInput")
    wq_d = nc.dram_tensor("wqt", [H, C], F32R, kind="ExternalInput")
    wk_d = nc.dram_tensor("wkt", [H, C], F32R, kind="ExternalInput")
    wv_d = nc.dram_tensor("wvt", [H, C], F32R, kind="ExternalInput")
    wo_d = nc.dram_tensor("wot", [C, H], F32R, kind="ExternalInput")
    bq_d = nc.dram_tensor("bq", [C, 1], F32, kind="ExternalInput")
    bk_d = nc.dram_tensor("bk", [C, 1], F32, kind="ExternalInput")
    bv_d = nc.dram_tensor("bv", [C, 1], F32, kind="ExternalInput")
    mk_d = nc.dram_tensor("maskbuf", [128, 896], F32R, kind="ExternalInput")
    id_d = nc.dram_tensor("ident", [128, 128], F32, kind="ExternalInput")
    on_d = nc.dram_tensor("ones16", [128, N_KC], F32R, kind="ExternalInput")
    out_d = nc.dram_tensor("out", [B, S, H], F32, kind="ExternalOutput")

    with tile.TileContext(nc) as tc:
        with (
            tc.tile_pool(name="const", bufs=1) as cp,
            tc.tile_pool(name="big", bufs=2) as bp,
            tc.tile_pool(name="work", bufs=2) as wp,
            tc.tile_pool(name="xs", bufs=12) as xp,
            tc.tile_pool(name="ps", bufs=1, space="PSUM") as ps,
            tc.tile_pool(name="psmix", bufs=2, space="PSUM") as pm,
        ):
            # ---- constants ----
            wq_s = cp.tile([128, H], F32R)
            wk_s = cp.tile([128, H], F32R)
            wv_s = cp.tile([128, H], F32R)
            wo_s = cp.tile([128, H], F32R)
            mk_s = cp.tile([128, 896], F32R)
            id_s = cp.tile([128, 128], F32)
            on_s = cp.tile([128, N_KC], F32R)
            bq_s = cp.tile([C, 1], F32)
            bk_s = cp.tile([C, 1], F32)
            bv_s = cp.tile([C, 1], F32)
            for w_s, w_d in ((wq_s, wq_d), (wk_s, wk_d), (wv_s, wv_d)):
                nc.scalar.dma_start(
                    w_s.rearrange("p (c d) -> p c d", d=128),
                    w_d.ap().rearrange("(c p) d -> p c d", p=128))
            nc.scalar.dma_start(wo_s[:], wo_d.ap())
            nc.scalar.dma_start(mk_s[:], mk_d.ap())
            nc.scalar.dma_start(id_s[:], id_d.ap())
            nc.scalar.dma_start(on_s[:], on_d.ap())
            nc.scalar.dma_start(bq_s[:], bq_d.ap())
            nc.scalar.dma_start(bk_s[:], bk_d.ap())
            nc.scalar.dma_start(bv_s[:], bv_d.ap())

            tiles = {}

            def emit_proj(b, halves=(0, 1)):
                # ---- projections: QT/KT [128, S] f32r, VT [128, S] f32 ----
                if b not in tiles:
                    qt = bp.tile([128, S], F32R, tag="qt", name=f"qt{b}")
                    kt = bp.tile([128, S], F32R, tag="kt", name=f"kt{b}")
                    vt = bp.tile([128, S], F32, tag="vt", name=f"vt{b}", bufs=1)
                    tiles[b] = {"qt": qt, "kt": kt, "vt": vt}
                qt, kt, vt = tiles[b]["qt"], tiles[b]["kt"], tiles[b]["vt"]
                if True:
                  for half in halves if "proj" in PHASES else []:
                    xts = []
                    for hc in range(N_HC):
                        hsl = slice(hc * 128, (hc + 1) * 128)
                        xt_t = xp.tile([128, 1024], F32R, tag="xt",
                                       name=f"x{b}_{half}_{hc}")
                        nc.sync.dma_start(
                            xt_t[:], xt_d.ap()[b, hsl, half * 1024:(half + 1) * 1024])
                        xts.append(xt_t)
                    for sth in range(2):
                        st = half * 2 + sth
                        ssl = slice(st * 512, (st + 1) * 512)
                        # sequential Q/K/V passes over resident x^T chunks: 2
                        # PSUM slots suffice (pipeline pass i+1 against copy i)
                        for w_s, bias, dst, pnm in ((wq_s, bq_s, qt, "q"),
                                                    (wk_s, bk_s, kt, "k"),
                                                    (wv_s, bv_s, vt, "v")):
                            pp = pm.tile([128, 512], F32, tag="mix",
                                         name=f"pp{pnm}{b}_{st}")
                            for hc in range(N_HC):
                                nc.tensor.matmul(
                                    pp[:], w_s[:, hc * 128:(hc + 1) * 128],
                                    xts[hc][:, sth * 512:(sth + 1) * 512],
                                    start=(hc == 0), stop=(hc == N_HC - 1))
                            nc.vector.tensor_scalar_add(dst[:, ssl], pp[:], bias[:])

            def emit_vtrans(b):
                # ---- V transpose: vn_h [128, 16*65] (ones col at 64 of each 65) ----
                vt = tiles[b]["vt"]
                vna = bp.tile([128, N_KC * (HD + 1)], F32R, tag="vna", name=f"vna{b}")
                vnb = bp.tile([128, N_KC * (HD + 1)], F32R, tag="vnb", name=f"vnb{b}")
                tiles[b]["vna"], tiles[b]["vnb"] = vna, vnb
                for h, vn in ((0, vna), (1, vnb)):
                    vn3 = vn.rearrange("p (c e) -> p c e", e=HD + 1)
                    nc.sync.dma_start(vn3[:, :, HD], on_d.ap())
                for c in range(N_KC) if "vtrans" in PHASES else []:
                    tp = pm.tile([128, 128], F32, tag="mix", name=f"tp{b}_{c}")
                    nc.tensor.transpose(tp[:], vt[:, c * 128:(c + 1) * 128], id_s[:])
                    nc.any.tensor_copy(vna[:, c * (HD + 1): c * (HD + 1) + HD],
                                       tp[:, 0:HD])
                    nc.any.tensor_copy(vnb[:, c * (HD + 1): c * (HD + 1) + HD],
                                       tp[:, HD:2 * HD])

            def emit_attn(b, jlo=0, jhi=N_QT):
                # ---- attention (transposed scores), both heads interleaved ----
                qt, kt = tiles[b]["qt"], tiles[b]["kt"]
                if "ctx" not in tiles[b]:
                    ctx = bp.tile([128, S], F32R, tag="ctx", name=f"ctx{b}")
                    tiles[b]["ctx"] = ctx
                ctx = tiles[b]["ctx"]
                vns = (tiles[b]["vna"], tiles[b]["vnb"])
                for j in range(jlo, jhi) if "attn" in PHASES else []:
                    qsl = slice(j * QT_W, (j + 1) * QT_W)
                    acc = ps.tile([128, 512], F32, tag="acc", name=f"acc{b}_{j}",
                                  bufs=2)
                    nc.vector.memset(acc[:], 0.0)
                    nkc = 2 * (j + 1)              # causal: k-chunks 0..nkc-1
                    n_sc = (nkc + 3) // 4
                    for sc in range(n_sc):
                        cs = [c for c in range(4 * sc, min(4 * sc + 4, nkc))]
                        sts, pts = [], []
                        for h in range(2):
                            st_h = ps.tile([128, 4 * QT_W], F32, tag=f"st{h}",
                                           name=f"st{h}_{b}_{j}_{sc}")
                            pt_h = wp.tile([128, 4 * QT_W], F32R, tag=f"pt{h}",
                                           name=f"pt{h}_{b}_{j}_{sc}", bufs=5)
                            sts.append(st_h)
                            pts.append(pt_h)
                        for c in cs:   # QK: heads adjacent -> row-group concurrency
                            for h in range(2):
                                hsl = slice(h * HD, (h + 1) * HD)
                                nc.tensor.matmul(
                                    sts[h][:, (c - 4 * sc) * QT_W:(c - 4 * sc + 1) * QT_W],
                                    kt[hsl, c * KC:(c + 1) * KC],
                                    qt[hsl, qsl],
                                    start=True, stop=True,
                                )
                        w = len(cs) * QT_W
                        for h in range(2):
                            nc.scalar.activation(pts[h][:, 0:w], sts[h][:, 0:w],
                                                 AF.Exp, scale=float(SCALE))
                        if sc == n_sc - 1:  # diagonal: mask last two k-chunks
                            for h in range(2):
                                for c in (nkc - 2, nkc - 1):
                                    mo = 384 - 128 * (c - 2 * j)  # o = 128*(c-2j)
                                    nc.gpsimd.tensor_mul(
                                        pts[h][:, (c - 4 * sc) * QT_W:(c - 4 * sc + 1) * QT_W],
                                        pts[h][:, (c - 4 * sc) * QT_W:(c - 4 * sc + 1) * QT_W],
                                        mk_s[:, mo:mo + QT_W],
                                    )
                        for c in cs:   # P@V (+ones rowsum row)
                            for h in range(2):
                                nc.tensor.matmul(
                                    acc[0:HD + 1, h * QT_W:(h + 1) * QT_W],
                                    vns[h][:, c * (HD + 1):(c + 1) * (HD + 1)],
                                    pts[h][:, (c - 4 * sc) * QT_W:(c - 4 * sc + 1) * QT_W],
                                    start=False, stop=(c == nkc - 1),
                                    skip_group_check=True,
                                )
                    # normalize: one recip over both heads' rowsum halves,
                    # partition-broadcast on the (idle) gpsimd, one fused mul
                    recip = wp.tile([1, 2 * QT_W], F32, tag="recip",
                                    name=f"rc{b}_{j}")
                    nc.vector.reciprocal(recip[:], acc[HD:HD + 1, :])
                    for h in range(2):
                        asl = slice(h * QT_W, (h + 1) * QT_W)
                        bc_sb = wp.tile([HD, QT_W], F32, tag="bcs",
                                        name=f"bcs{b}_{j}_{h}", bufs=4)
                        nc.gpsimd.partition_broadcast(bc_sb[:], recip[0:1, asl])
                        nc.any.tensor_mul(ctx[h * HD:(h + 1) * HD, qsl],
                                          acc[0:HD, asl], bc_sb[:])

            def emit_oproj(b):
                ctx = tiles[b]["ctx"]
                for qp in range(S // 256) if "oproj" in PHASES else []:
                    osb = wp.tile([128, 2048], F32, tag="osb", name=f"ob{b}_{qp}")
                    for sub in range(2):
                        qc = 2 * qp + sub
                        for half in range(2):
                            osl = slice(half * 512, (half + 1) * 512)
                            op = pm.tile([128, 512], F32, tag="mix",
                                         name=f"op{b}_{qc}_{half}")
                            nc.tensor.matmul(op[:], ctx[:, qc * 128:(qc + 1) * 128],
                                             wo_s[:, osl], start=True, stop=True)
                            nc.vector.tensor_copy(
                                osb[:, sub * 1024 + half * 512:
                                    sub * 1024 + (half + 1) * 512], op[:])
                    nc.sync.dma_start(
                        out_d.ap()[b, qp * 256:(qp + 1) * 256, :]
                        .rearrange("(g q) o -> q g o", g=2),
                        osb.rearrange("p (g o) -> p g o", g=2))

            # software-pipelined emission: batch b+1's projection halves are
            # interleaved into batch b's (ACT-gated) attention j-loop so PE
            # always has prioritized fill work; the heavier fill (half 1 +
            # V-transpose) lands before the large causal j-tiles
            emit_proj(0)
            emit_vtrans(0)
            for b in range(B):
                if b + 1 < B:
                    emit_proj(b + 1, halves=(0,))
                emit_attn(b, 0, 4)
                if b + 1 < B:
                    emit_proj(b + 1, halves=(1,))
                    emit_vtrans(b + 1)
                emit_attn(b, 4, N_QT)
                emit_oproj(b)

                # ---- output projection (partial over this core's channels) ----

    nc.compile()
    return nc


def _get_nc():
    if "nc" not in _CACHE:
        _CACHE["nc"] = _build_nc()
    return _CACHE["nc"]


def make_in_maps(x, Wq, bq, Wk, bk, Wv, bv, Wo):
    """Host-side sharding: returns per-core input dicts."""
    xt = np.ascontiguousarray(np.transpose(np.asarray(x, np.float32), (0, 2, 1)))
    mask = (np.arange(896, dtype=np.int64)[None, :]
            >= (np.arange(128, dtype=np.int64)[:, None] + 384)).astype(np.float32)
    ident = np.eye(128, dtype=np.float32)
    ones16 = np.ones((128, N_KC), dtype=np.float32)
    in_maps = []
    for i in range(NCORES):
        r = slice(i * C, (i + 1) * C)
        in_maps.append({
            "xt": xt,
            "wqt": np.ascontiguousarray(np.asarray(Wq, np.float32)[r, :].T),
            "wkt": np.ascontiguousarray(np.asarray(Wk, np.float32)[r, :].T),
            "wvt": np.ascontiguousarray(np.asarray(Wv, np.float32)[r, :].T),
            "wot": np.ascontiguousarray(np.asarray(Wo, np.float32)[:, r].T),
            "bq": np.asarray(bq, np.float32)[r].reshape(C, 1),
            "bk": np.asarray(bk, np.float32)[r].reshape(C, 1),
            "bv": np.asarray(bv, np.float32)[r].reshape(C, 1),
            "maskbuf": mask,
            "ident": ident,
            "ones16": ones16,
        })
    return in_maps


def run_cores(in_maps):
    nc = _get_nc()
    res = run_bass_kernel_spmd(nc, in_maps, core_ids=list(range(NCORES)))
    return [r["out"] for r in res.results]


def kernel(x, mask, Wq, bq, Wk, bk, Wv, bv, Wo, bo):
    in_maps = make_in_maps(x, Wq, bq, Wk, bk, Wv, bv, Wo)
    partials = run_cores(in_maps)
    out = partials[0]
    for p in partials[1:]:
        out = out + p
    return (out + np.asarray(bo, np.float32)[None, None, :]).astype(np.float32)
